# revision 4
# baseline (speedup 1.0000x reference)
"""Single-head causal attention on 8 Trainium2 NeuronCores.

Sharding: data-parallel over batch (4 batches x 2 cores), sequence-parallel
within each pair. Core c handles batch b=c//2 and query tiles g = 2j + h
(h=c%2, j=0..7 local slot index) -- interleaved so both cores of a pair do
identical causal work (SPMD: one program, per-core data only).

Per-core compute (S=2048, D=1024, P=128):
  qT[e,q]  = Wq^T xq          (fp32r matmuls, fp32r storage)
  kT[e,k]  = Wk^T xT          (fp32r)
  v[k,e]   = xT^T Wv          (bf16 storage)
  slot j (local q-tile, global tile g=2j+h) computes keys [0, 256(j+1)):
    S[q,k]   = exp(scale*(qT^T kT + maskS))   row-summed on ScalarE
    attw     = S / rowsum  -> DRAM (skipped cols stay zero)
  PT[k,q]  = exp(scale*(kT^T qT + maskT))     (bf16) -- transposed scores
  O[q,e]   = (PT^T v) / rowsum -> DRAM        (bf16 matmuls)

The causal mask is additive input data (triangular / full / zero 128-wide
slabs) so both cores run the same instruction stream.
"""

from contextlib import ExitStack

import numpy as np

import concourse.bass as bass
import concourse.tile as tile
from concourse import bacc, mybir
from concourse.bass_utils import run_bass_kernel_spmd

S = 2048
D = 1024
B = 4
P = 128
N_CORES = 8
N_SLOTS = 8  # local q tiles per core
N_KT = 16  # k tiles
F32 = mybir.dt.float32
F32R = mybir.dt.float32r
BF16 = mybir.dt.bfloat16
F16 = mybir.dt.float16
PV_DT = F16  # storage dtype for PT and v (the AV-matmul operands)
EXPF = mybir.ActivationFunctionType.Exp
SCALE = 1.0 / 32.0  # 1/sqrt(D)
NEG = -1.0e9


def _col_blocks(total, width=512):
    """Split [0, total) into blocks of at most `width`."""
    out = []
    c = 0
    while c < total:
        w = min(width, total - c)
        out.append((c, w))
        c += w
    return out


def build_kernel(loop_iters: int = 1):
    """Emit the SPMD attention kernel. loop_iters>1 wraps the body in a
    hardware For_i loop (benchmarking only)."""
    nc = bacc.Bacc("TRN2", target_bir_lowering=False, debug=False)

    xt = nc.dram_tensor("xt", [D, S], F32R, kind="ExternalInput")
    xq = nc.dram_tensor("xq", [D, D], F32R, kind="ExternalInput")
    wq = nc.dram_tensor("wq", [D, D], F32R, kind="ExternalInput")
    wk = nc.dram_tensor("wk", [D, D], F32R, kind="ExternalInput")
    wv = nc.dram_tensor("wv", [D, D], F32R, kind="ExternalInput")
    ms = nc.dram_tensor("ms", [N_SLOTS * P, 256], F32, kind="ExternalInput")
    mt = nc.dram_tensor("mt", [N_KT * P, P], F32, kind="ExternalInput")
    attw = nc.dram_tensor("attw", [N_SLOTS * P, S], F32, kind="ExternalOutput")
    outp = nc.dram_tensor("outp", [N_SLOTS * P, D], F32, kind="ExternalOutput")

    with tile.TileContext(nc) as tc, ExitStack() as ctx:
        if loop_iters > 1:
            loop = ctx.enter_context(tc.For_i(0, loop_iters, 1))  # noqa: F841

        psum = ctx.enter_context(tc.tile_pool(name="psum", bufs=6, space="PSUM"))
        persist = ctx.enter_context(tc.tile_pool(name="persist", bufs=1))

        qT = [persist.tile([P, D], F32R, tag=f"qT{e}", name=f"qT{e}") for e in range(8)]
        kT = [persist.tile([P, S], F32R, tag=f"kT{e}", name=f"kT{e}") for e in range(8)]
        vv = [persist.tile([P, D], PV_DT, tag=f"v{t}", name=f"v{t}") for t in range(N_KT)]
        rinv = [persist.tile([P, 1], F32, tag=f"rinv{j}", name=f"rinv{j}") for j in range(N_SLOTS)]

        # ---------------- phase A: projections ----------------
        with (
            tc.tile_pool(name="wpool", bufs=1) as wpool,
            tc.tile_pool(name="xstream", bufs=2) as xs,
        ):
            # qT[e, q] = sum_d Wq[d, e]^T xq[d, q]
            w_s = [wpool.tile([P, D], F32R, tag=f"w{d}", name=f"w{d}") for d in range(8)]
            for d in range(8):
                nc.sync.dma_start(out=w_s[d][:], in_=wq[d * P : (d + 1) * P, :])
            for qb in range(2):
                xq_b = [xs.tile([P, 512], F32R, tag=f"x{d}", name=f"x{d}") for d in range(8)]
                for d in range(8):
                    nc.sync.dma_start(
                        out=xq_b[d][:],
                        in_=xq[d * P : (d + 1) * P, qb * 512 : (qb + 1) * 512],
                    )
                for e in range(8):
                    ps = psum.tile([P, 512], F32, tag="ps")
                    for d in range(8):
                        nc.tensor.matmul(
                            ps[:],
                            w_s[d][:, e * P : (e + 1) * P],
                            xq_b[d][:],
                            start=(d == 0),
                            stop=(d == 7),
                        )
                    nc.vector.tensor_copy(qT[e][:, qb * 512 : (qb + 1) * 512], ps[:])

            # kT[e, k] = sum_d Wk[d, e]^T xT[d, k]
            w_s = [wpool.tile([P, D], F32R, tag=f"w{d}", name=f"w{d}") for d in range(8)]
            for d in range(8):
                nc.sync.dma_start(out=w_s[d][:], in_=wk[d * P : (d + 1) * P, :])
            for kb in range(4):
                xt_b = [xs.tile([P, 512], F32R, tag=f"x{d}", name=f"x{d}") for d in range(8)]
                for d in range(8):
                    nc.sync.dma_start(
                        out=xt_b[d][:],
                        in_=xt[d * P : (d + 1) * P, kb * 512 : (kb + 1) * 512],
                    )
                for e in range(8):
                    ps = psum.tile([P, 512], F32, tag="ps")
                    for d in range(8):
                        nc.tensor.matmul(
                            ps[:],
                            w_s[d][:, e * P : (e + 1) * P],
                            xt_b[d][:],
                            start=(d == 0),
                            stop=(d == 7),
                        )
                    nc.vector.tensor_copy(kT[e][:, kb * 512 : (kb + 1) * 512], ps[:])

            # v[k, e] = sum_d xT[d, k]^T Wv[d, e]
            w_s = [wpool.tile([P, D], F32R, tag=f"w{d}", name=f"w{d}") for d in range(8)]
            for d in range(8):
                nc.sync.dma_start(out=w_s[d][:], in_=wv[d * P : (d + 1) * P, :])
            for kt in range(N_KT):
                xt_t = [xs.tile([P, P], F32R, tag=f"xk{d}", name=f"xk{d}") for d in range(8)]
                for d in range(8):
                    nc.sync.dma_start(
                        out=xt_t[d][:],
                        in_=xt[d * P : (d + 1) * P, kt * P : (kt + 1) * P],
                    )
                for eb in range(2):
                    ps = psum.tile([P, 512], F32, tag="ps")
                    for d in range(8):
                        nc.tensor.matmul(
                            ps[:],
                            xt_t[d][:],
                            w_s[d][:, eb * 512 : (eb + 1) * 512],
                            start=(d == 0),
                            stop=(d == 7),
                        )
                    nc.vector.tensor_copy(vv[kt][:, eb * 512 : (eb + 1) * 512], ps[:])

        # ---------------- phase B: scores + softmax + attw ----------------
        spool = ctx.enter_context(tc.tile_pool(name="spool", bufs=2))
        mpool = ctx.enter_context(tc.tile_pool(name="masks", bufs=2))
        rpool = ctx.enter_context(tc.tile_pool(name="rstats", bufs=2))

        for j in range(N_SLOTS):
            ej = 256 * (j + 1)  # causal k-extent of this slot
            ms_t = mpool.tile([P, 256], F32, tag="ms")
            nc.sync.dma_start(out=ms_t[:], in_=ms[j * P : (j + 1) * P, :])
            s_t = spool.tile([P, S], F32, tag="S")
            rs_blocks = []
            for bi, (c0, w) in enumerate(_col_blocks(ej)):
                ps = psum.tile([P, 512], F32, tag="ps")
                for e in range(8):
                    nc.tensor.matmul(
                        ps[:, :w],
                        qT[e][:, j * P : (j + 1) * P],
                        kT[e][:, c0 : c0 + w],
                        start=(e == 0),
                        stop=(e == 7),
                    )
                if c0 + w == ej:  # causal boundary: mask last 256 cols
                    nc.vector.tensor_add(
                        ps[:, w - 256 : w], ps[:, w - 256 : w], ms_t[:]
                    )
                rs = rpool.tile([P, 1], F32, tag=f"rs{bi}")
                nc.scalar.activation(
                    s_t[:, c0 : c0 + w], ps[:, :w], EXPF, scale=SCALE, accum_out=rs[:]
                )
                rs_blocks.append(rs)
            acc = rs_blocks[0]
            for r in rs_blocks[1:]:
                nc.vector.tensor_add(acc[:], acc[:], r[:])
            nc.vector.reciprocal(rinv[j][:], acc[:])
            nc.vector.tensor_scalar_mul(s_t[:, :ej], s_t[:, :ej], rinv[j][:])
            nc.sync.dma_start(out=attw[j * P : (j + 1) * P, 0:ej], in_=s_t[:, :ej])

        # ---------------- phase C: transposed scores PT ----------------
        ptpool = ctx.enter_context(tc.tile_pool(name="ptpool", bufs=1))
        PT = [
            ptpool.tile([P, D - P * (kt // 2)], PV_DT, tag=f"PT{kt}", name=f"PT{kt}")
            for kt in range(N_KT)
        ]
        for kt in range(N_KT):
            qc0 = P * (kt // 2)  # first local q column this k-tile can see
            wdt = D - qc0
            mt_t = mpool.tile([P, P], F32, tag="mt")
            nc.sync.dma_start(out=mt_t[:], in_=mt[kt * P : (kt + 1) * P, :])
            for lo, w in _col_blocks(wdt):
                ps = psum.tile([P, 512], F32, tag="ps")
                for e in range(8):
                    nc.tensor.matmul(
                        ps[:, :w],
                        kT[e][:, kt * P : (kt + 1) * P],
                        qT[e][:, qc0 + lo : qc0 + lo + w],
                        start=(e == 0),
                        stop=(e == 7),
                    )
                if lo == 0:  # causal boundary lives in the first 128 cols
                    nc.vector.tensor_add(ps[:, 0:P], ps[:, 0:P], mt_t[:])
                nc.scalar.activation(PT[kt][:, lo : lo + w], ps[:, :w], EXPF, scale=SCALE)

        # ---------------- phase D: output O = PT^T v / rowsum ----------------
        opool = ctx.enter_context(tc.tile_pool(name="opool", bufs=2))
        for j in range(N_SLOTS):
            n_kt = 2 * j + 2  # causal: k tiles 0..2j+1
            o_t = opool.tile([P, D], F32, tag="o")
            for eb in range(2):
                ps = psum.tile([P, 512], F32, tag="ps")
                for kt in range(n_kt):
                    lo = j * P - P * (kt // 2)
                    nc.tensor.matmul(
                        ps[:],
                        PT[kt][:, lo : lo + P],
                        vv[kt][:, eb * 512 : (eb + 1) * 512],
                        start=(kt == 0),
                        stop=(kt == n_kt - 1),
                    )
                nc.vector.tensor_scalar_mul(
                    o_t[:, eb * 512 : (eb + 1) * 512], ps[:], rinv[j][:]
                )
            nc.sync.dma_start(out=outp[j * P : (j + 1) * P, :], in_=o_t[:])

    nc.finalize()
    return nc


def _masks(h: int):
    """Additive causal masks for query-half h (core-dependent data)."""
    tri = np.triu(np.full((P, P), NEG, np.float32), 1)  # [i,j]=NEG where j>i
    tri_t = np.ascontiguousarray(tri.T)  # [k,q]=NEG where k>q
    full = np.full((P, P), NEG, np.float32)
    zero = np.zeros((P, P), np.float32)
    # maskS: slot-local last 256 score cols (k tiles 2j, 2j+1 vs q tile 2j+h)
    slab = np.hstack([tri, full]) if h == 0 else np.hstack([zero, tri])
    ms_h = np.tile(slab, (N_SLOTS, 1))
    # maskT: first 128 PT cols of k-tile kt (q tile 2*(kt//2)+h)
    rows = []
    for kt in range(N_KT):
        if h == 0:
            rows.append(tri_t if kt % 2 == 0 else full)
        else:
            rows.append(tri_t if kt % 2 == 1 else zero)
    mt_h = np.vstack(rows)
    return np.ascontiguousarray(ms_h), np.ascontiguousarray(mt_h)


_CACHE: dict = {}


def _make_in_maps(x, Wq, Wk, Wv):
    x = np.ascontiguousarray(np.asarray(x, dtype=np.float32))
    Wq = np.ascontiguousarray(np.asarray(Wq, dtype=np.float32))
    Wk = np.ascontiguousarray(np.asarray(Wk, dtype=np.float32))
    Wv = np.ascontiguousarray(np.asarray(Wv, dtype=np.float32))
    masks = {h: _masks(h) for h in (0, 1)}
    xts = {b: np.ascontiguousarray(x[b].T) for b in range(B)}
    in_maps = []
    for c in range(N_CORES):
        b, h = c // 2, c % 2
        xtb = xts[b]
        xq_c = np.ascontiguousarray(
            np.concatenate(
                [
                    xtb[:, (2 * j + h) * P : (2 * j + h) * P + P]
                    for j in range(N_SLOTS)
                ],
                axis=1,
            )
        )
        ms_h, mt_h = masks[h]
        in_maps.append(
            {
                "xt": xtb,
                "xq": xq_c,
                "wq": Wq,
                "wk": Wk,
                "wv": Wv,
                "ms": ms_h,
                "mt": mt_h,
            }
        )
    return in_maps


def _assemble(results):
    att_w = np.zeros((B, S, S), np.float32)
    att_o = np.zeros((B, S, D), np.float32)
    for c in range(N_CORES):
        b, h = c // 2, c % 2
        rw = results[c]["attw"]
        ro = results[c]["outp"]
        for j in range(N_SLOTS):
            g = 2 * j + h
            att_w[b, g * P : (g + 1) * P, :] = rw[j * P : (j + 1) * P, :]
            att_o[b, g * P : (g + 1) * P, :] = ro[j * P : (j + 1) * P, :]
    return att_o, att_w


def kernel(x, Wq, Wk, Wv):
    if "nc" not in _CACHE:
        _CACHE["nc"] = build_kernel()
    nc = _CACHE["nc"]
    in_maps = _make_in_maps(x, Wq, Wk, Wv)
    res = run_bass_kernel_spmd(nc, in_maps, list(range(N_CORES))).results
    return _assemble(res)


# revision 13
# speedup vs baseline: 1.1767x; 1.1767x over previous
"""Single-head causal attention on 8 Trainium2 NeuronCores.

Sharding: data-parallel over batch (4 batches x 2 cores), sequence-parallel
within each pair. Core c handles batch b=c//2 and query tiles g = 2j + h
(h=c%2, j=0..7 local slot index) -- interleaved so both cores of a pair do
identical causal work (SPMD: one program, per-core data only).

Per-core compute (S=2048, D=1024, P=128):
  qT[e,q]  = Wq^T xq          (fp32r matmuls, fp32r storage)
  kT[e,k]  = Wk^T xT          (fp32r)
  v[k,e]   = xT^T Wv          (fp16 storage)
  slot j (local q-tile, global tile g=2j+h) computes keys [0, 256(j+1)):
    S[q,k]   = exp(scale*(qT^T kT + maskS))   row-summed on ScalarE
    PT[k,q] <- PE-transpose of the S tiles (unnormalized, fp16)
    attw     = S / rowsum  -> DRAM (skipped cols stay zero)
  O[q,e]   = (PT^T v) / rowsum -> DRAM        (fp16 matmuls)

The causal mask is additive input data (triangular / full / zero 128-wide
slabs) so both cores run the same instruction stream.
"""

from contextlib import ExitStack

import numpy as np

import concourse.bass as bass
import concourse.tile as tile
from concourse import bacc, mybir
from concourse.bass_utils import run_bass_kernel_spmd

S = 2048
D = 1024
B = 4
P = 128
N_CORES = 8
N_SLOTS = 8  # local q tiles per core
N_KT = 16  # k tiles
F32 = mybir.dt.float32
F32R = mybir.dt.float32r
F16 = mybir.dt.float16
PV_DT = F16  # storage dtype for PT and v (the AV-matmul operands)
EXPF = mybir.ActivationFunctionType.Exp
SCALE = 1.0 / 32.0  # 1/sqrt(D)
NEG = -1.0e9


def _col_blocks(total, width=512):
    """Split [0, total) into blocks of at most `width`."""
    out = []
    c = 0
    while c < total:
        w = min(width, total - c)
        out.append((c, w))
        c += w
    return out


def build_kernel(loop_iters: int = 1):
    """Emit the SPMD attention kernel. loop_iters>1 wraps the body in a
    hardware For_i loop (benchmarking only)."""
    nc = bacc.Bacc("TRN2", target_bir_lowering=False, debug=False)

    xt = nc.dram_tensor("xt", [D, S], F32R, kind="ExternalInput")
    xq = nc.dram_tensor("xq", [D, D], F32R, kind="ExternalInput")
    wq = nc.dram_tensor("wq", [D, D], F32R, kind="ExternalInput")
    wk = nc.dram_tensor("wk", [D, D], F32R, kind="ExternalInput")
    wv = nc.dram_tensor("wv", [D, D], F32R, kind="ExternalInput")
    ms = nc.dram_tensor("ms", [N_SLOTS * P, 256], F32, kind="ExternalInput")
    ident = nc.dram_tensor("ident", [P, P], F32, kind="ExternalInput")
    attw = nc.dram_tensor("attw", [N_SLOTS * P, S], F32, kind="ExternalOutput")
    outp = nc.dram_tensor("outp", [N_SLOTS * P, D], F32, kind="ExternalOutput")

    with tile.TileContext(nc) as tc, ExitStack() as ctx:
        if loop_iters > 1:
            loop = ctx.enter_context(tc.For_i(0, loop_iters, 1))  # noqa: F841

        psum = ctx.enter_context(tc.tile_pool(name="psum", bufs=5, space="PSUM"))
        persist = ctx.enter_context(tc.tile_pool(name="persist", bufs=1))

        qT = [persist.tile([P, D], F32R, tag=f"qT{e}", name=f"qT{e}") for e in range(8)]
        kT = [persist.tile([P, S], F32R, tag=f"kT{e}", name=f"kT{e}") for e in range(8)]
        vv = [persist.tile([P, D], PV_DT, tag=f"v{t}", name=f"v{t}") for t in range(N_KT)]
        rinv = [
            persist.tile([P, 1], F32, tag=f"rinv{j}", name=f"rinv{j}")
            for j in range(N_SLOTS)
        ]
        id_t = persist.tile([P, P], F32, tag="ident", name="id_t")
        nc.sync.dma_start(out=id_t[:], in_=ident[:, :])

        # ---------------- phase A: projections ----------------
        with (
            tc.tile_pool(name="wpool", bufs=1) as wpool,
            tc.tile_pool(name="xstream", bufs=2) as xs,
        ):
            # qT[e, q] = sum_d Wq[d, e]^T xq[d, q].  First data on the wire:
            # the qb=0 x-block plus the first weight column chunk.
            w_s = [wpool.tile([P, D], F32R, tag=f"w{d}", name=f"w{d}") for d in range(8)]
            xq_b0 = [xs.tile([P, 512], F32R, tag=f"x{d}", name=f"x{d}") for d in range(8)]
            for d in range(8):
                nc.sync.dma_start(out=xq_b0[d][:], in_=xq[d * P : (d + 1) * P, 0:512])
            for d in range(8):
                nc.sync.dma_start(out=w_s[d][:, 0:P], in_=wq[d * P : (d + 1) * P, 0:P])
            for d in range(8):
                nc.sync.dma_start(out=w_s[d][:, P:D], in_=wq[d * P : (d + 1) * P, P:D])
            for qb in range(2):
                if qb == 0:
                    xq_b = xq_b0
                else:
                    xq_b = [
                        xs.tile([P, 512], F32R, tag=f"x{d}", name=f"x{d}")
                        for d in range(8)
                    ]
                    for d in range(8):
                        nc.sync.dma_start(
                            out=xq_b[d][:],
                            in_=xq[d * P : (d + 1) * P, qb * 512 : (qb + 1) * 512],
                        )
                for e in range(8):
                    ps = psum.tile([P, 512], F32, tag="ps", bufs=5)
                    for d in range(8):
                        nc.tensor.matmul(
                            ps[:],
                            w_s[d][:, e * P : (e + 1) * P],
                            xq_b[d][:],
                            start=(d == 0),
                            stop=(d == 7),
                        )
                    nc.vector.tensor_copy(qT[e][:, qb * 512 : (qb + 1) * 512], ps[:])

            # kT[e, k] = sum_d Wk[d, e]^T xT[d, k]
            w_s = [wpool.tile([P, D], F32R, tag=f"w{d}", name=f"w{d}") for d in range(8)]
            for d in range(8):
                nc.sync.dma_start(out=w_s[d][:], in_=wk[d * P : (d + 1) * P, :])
            for kb in range(4):
                xt_b = [
                    xs.tile([P, 512], F32R, tag=f"x{d}", name=f"x{d}") for d in range(8)
                ]
                for d in range(8):
                    nc.sync.dma_start(
                        out=xt_b[d][:],
                        in_=xt[d * P : (d + 1) * P, kb * 512 : (kb + 1) * 512],
                    )
                for e in range(8):
                    ps = psum.tile([P, 512], F32, tag="ps", bufs=5)
                    for d in range(8):
                        nc.tensor.matmul(
                            ps[:],
                            w_s[d][:, e * P : (e + 1) * P],
                            xt_b[d][:],
                            start=(d == 0),
                            stop=(d == 7),
                        )
                    nc.vector.tensor_copy(kT[e][:, kb * 512 : (kb + 1) * 512], ps[:])

            # v[k, e] = sum_d xT[d, k]^T Wv[d, e]
            w_s = [wpool.tile([P, D], F32R, tag=f"w{d}", name=f"w{d}") for d in range(8)]
            for d in range(8):
                nc.sync.dma_start(out=w_s[d][:], in_=wv[d * P : (d + 1) * P, :])
            for kt in range(N_KT):
                xt_t = [
                    xs.tile([P, P], F32R, tag=f"x{d}", name=f"xk{d}") for d in range(8)
                ]
                for d in range(8):
                    nc.sync.dma_start(
                        out=xt_t[d][:],
                        in_=xt[d * P : (d + 1) * P, kt * P : (kt + 1) * P],
                    )
                for eb in range(2):
                    ps = psum.tile([P, 512], F32, tag="ps", bufs=5)
                    for d in range(8):
                        nc.tensor.matmul(
                            ps[:],
                            xt_t[d][:],
                            w_s[d][:, eb * 512 : (eb + 1) * 512],
                            start=(d == 0),
                            stop=(d == 7),
                        )
                    nc.vector.tensor_copy(vv[kt][:, eb * 512 : (eb + 1) * 512], ps[:])

        # -------- phase B: scores + softmax + attw + PT via transpose -----
        spool = ctx.enter_context(tc.tile_pool(name="spool", bufs=2))
        mpool = ctx.enter_context(tc.tile_pool(name="masks", bufs=2))
        rpool = ctx.enter_context(tc.tile_pool(name="rstats", bufs=2))
        ptpool = ctx.enter_context(tc.tile_pool(name="ptpool", bufs=1))
        PT = [
            ptpool.tile([P, D - P * (kt // 2)], PV_DT, tag=f"PT{kt}", name=f"PT{kt}")
            for kt in range(N_KT)
        ]

        for j in range(N_SLOTS):
            ej = 256 * (j + 1)  # causal k-extent of this slot
            ms_t = mpool.tile([P, 256], F32, tag="ms")
            nc.sync.dma_start(out=ms_t[:], in_=ms[j * P : (j + 1) * P, :])
            s_t = spool.tile([P, S], F32, tag="S")
            rs_blocks = []
            for bi, (c0, w) in enumerate(_col_blocks(ej)):
                ps = psum.tile([P, 512], F32, tag="ps", bufs=5)
                for e in range(8):
                    nc.tensor.matmul(
                        ps[:, :w],
                        qT[e][:, j * P : (j + 1) * P],
                        kT[e][:, c0 : c0 + w],
                        start=(e == 0),
                        stop=(e == 7),
                    )
                if c0 + w == ej:  # causal boundary: mask last 256 cols
                    nc.vector.tensor_add(
                        ps[:, w - 256 : w], ps[:, w - 256 : w], ms_t[:]
                    )
                rs = rpool.tile([P, 1], F32, tag=f"rs{bi}")
                nc.scalar.activation(
                    s_t[:, c0 : c0 + w], ps[:, :w], EXPF, scale=SCALE, accum_out=rs[:]
                )
                rs_blocks.append(rs)
            # PT[kt][k, q-slot j] = S[q-slot j, kt]^T (unnormalized)
            for kt in range(2 * j + 2):
                lo = j * P - P * (kt // 2)
                tp = psum.tile([P, P], F32, tag="tps", bufs=3, name="tp")
                nc.tensor.transpose(tp[:], s_t[:, kt * P : (kt + 1) * P], id_t[:])
                nc.vector.tensor_copy(PT[kt][:, lo : lo + P], tp[:])
            acc = rs_blocks[0]
            for r in rs_blocks[1:]:
                nc.vector.tensor_add(acc[:], acc[:], r[:])
            nc.vector.reciprocal(rinv[j][:], acc[:])
            nc.vector.tensor_scalar_mul(s_t[:, :ej], s_t[:, :ej], rinv[j][:])
            nc.sync.dma_start(out=attw[j * P : (j + 1) * P, 0:ej], in_=s_t[:, :ej])

        # ---------------- phase D: output O = PT^T v / rowsum ------------
        opool = ctx.enter_context(tc.tile_pool(name="opool", bufs=2))
        for j in range(N_SLOTS):
            n_kt = 2 * j + 2  # causal: k tiles 0..2j+1
            o_t = opool.tile([P, D], F32, tag="o")
            for eb in range(2):
                ps = psum.tile([P, 512], F32, tag="ps", bufs=5)
                for kt in range(n_kt):
                    lo = j * P - P * (kt // 2)
                    nc.tensor.matmul(
                        ps[:],
                        PT[kt][:, lo : lo + P],
                        vv[kt][:, eb * 512 : (eb + 1) * 512],
                        start=(kt == 0),
                        stop=(kt == n_kt - 1),
                    )
                nc.vector.tensor_scalar_mul(
                    o_t[:, eb * 512 : (eb + 1) * 512], ps[:], rinv[j][:]
                )
            nc.sync.dma_start(out=outp[j * P : (j + 1) * P, :], in_=o_t[:])

    nc.finalize()
    return nc


def _masks(h: int):
    """Additive causal maskS for query-half h (core-dependent data)."""
    tri = np.triu(np.full((P, P), NEG, np.float32), 1)  # [i,j]=NEG where j>i
    full = np.full((P, P), NEG, np.float32)
    zero = np.zeros((P, P), np.float32)
    # maskS: slot-local last 256 score cols (k tiles 2j, 2j+1 vs q tile 2j+h)
    slab = np.hstack([tri, full]) if h == 0 else np.hstack([zero, tri])
    return np.ascontiguousarray(np.tile(slab, (N_SLOTS, 1)))


_CACHE: dict = {}


def _make_in_maps(x, Wq, Wk, Wv):
    x = np.ascontiguousarray(np.asarray(x, dtype=np.float32))
    Wq = np.ascontiguousarray(np.asarray(Wq, dtype=np.float32))
    Wk = np.ascontiguousarray(np.asarray(Wk, dtype=np.float32))
    Wv = np.ascontiguousarray(np.asarray(Wv, dtype=np.float32))
    masks = {h: _masks(h) for h in (0, 1)}
    eye = np.ascontiguousarray(np.eye(P, dtype=np.float32))
    xts = {b: np.ascontiguousarray(x[b].T) for b in range(B)}
    in_maps = []
    for c in range(N_CORES):
        b, h = c // 2, c % 2
        xtb = xts[b]
        xq_c = np.ascontiguousarray(
            np.concatenate(
                [
                    xtb[:, (2 * j + h) * P : (2 * j + h) * P + P]
                    for j in range(N_SLOTS)
                ],
                axis=1,
            )
        )
        in_maps.append(
            {
                "xt": xtb,
                "xq": xq_c,
                "wq": Wq,
                "wk": Wk,
                "wv": Wv,
                "ms": masks[h],
                "ident": eye,
            }
        )
    return in_maps


def _assemble(results):
    att_w = np.zeros((B, S, S), np.float32)
    att_o = np.zeros((B, S, D), np.float32)
    for c in range(N_CORES):
        b, h = c // 2, c % 2
        rw = results[c]["attw"]
        ro = results[c]["outp"]
        for j in range(N_SLOTS):
            g = 2 * j + h
            att_w[b, g * P : (g + 1) * P, :] = rw[j * P : (j + 1) * P, :]
            att_o[b, g * P : (g + 1) * P, :] = ro[j * P : (j + 1) * P, :]
    return att_o, att_w


def kernel(x, Wq, Wk, Wv):
    if "nc" not in _CACHE:
        _CACHE["nc"] = build_kernel()
    nc = _CACHE["nc"]
    in_maps = _make_in_maps(x, Wq, Wk, Wv)
    res = run_bass_kernel_spmd(nc, in_maps, list(range(N_CORES))).results
    return _assemble(res)
